# revision 9
# baseline (speedup 1.0000x reference)
"""Trainium2 Bass kernel for nn_ConditionalFeedForward (MoE top-2 FFN).

Strategy: expert-parallel across the 8 NeuronCores — expert e lives on core e.
Routing/gather/scatter (pure index bookkeeping) happens on the host; all FLOPs
(both GEMMs + SiLU) run on device.

Per core (expert e), with C = per-expert token capacity:
    h13T = w13[e] @ xgT          # [2I, C], accumulation over D in PSUM
    hT   = silu(gate) * up       # [I, C]
    outT = w2[e] @ hT            # [D, C]
Everything is kept transposed ([feature, token]) so both GEMMs use the weight
as the stationary operand and never need an on-device transpose.

Performance notes (from NTFF profiling):
  - PSUM tiles are padded to full 2KB banks: two accumulation chains sharing
    one bank cost ~26ns on every matmul issue (144 vs the 118ns floor).
  - All weights are SBUF-resident (~14MB).  Weight DMAs are issued up-front
    in consumption order on the sync HWDGE queue; loop-carried WAR deps
    (a tile is reloadable once the previous rep finished reading it)
    self-regulate into a full iteration of prefetch.
  - Weights ship as a few big slab DMAs (not 24 per-panel ones): each
    DMA_DIRECT2D costs ~650ns of issuing-engine NX time, which otherwise
    serializes into a ~16us refill after every For_i loop barrier.
  - The 8 per-tile output DMAs are batched into one [P, MO*C] store: tiny
    552B-per-partition descriptors otherwise pollute the SDMA engines'
    packet round-robin.
  - Output DMA rides the scalar HWDGE queue so it never blocks weight
    prefetch; output is cast to bf16 by the DVE copy.
  - PSUM: psg/psu double-buffered, ps2 triple, output staging double -
    the out-DMA completion latency (~2-3us HBM write receipt) otherwise
    backpressures GEMM2 via the PSUM-bank recycle chain.
  - For_i back-edge costs ~6-13us (all-engine barrier + DMA re-issue + PE
    HAM re-throttle), so `unroll` bodies run per hw-loop iteration and a
    dummy-matmul warm block covers the post-barrier refill window.
"""

import math
from contextlib import ExitStack

import ml_dtypes
import numpy as np

import concourse.bass as bass
import concourse.mybir as mybir
import concourse.tile as tile
from concourse import bacc
from concourse.bass_utils import run_bass_kernel_spmd

# Problem shape (hardcoded per harness contract).
E = 8          # experts == cores
D = 1024       # model dim
I = 2048       # intermediate dim
I2 = 2 * I     # fused gate+up rows of w13
P = 128        # SBUF partitions
KD = D // P    # 8 k-tiles over D
MP = I // P    # 16 gate/up pair panels
MO = D // P    # 8 output row tiles
KI = I // P    # 16 k-tiles over I

PANW13 = KD * 2 * P      # w13 pair-panel width in elements (4096)
PANW2 = KI * P           # w2 out-panel width in elements (2048)
NHEAD = 2                # leading w13 panels shipped individually
SLAB13 = (MP - NHEAD) // 2   # 7 panels per w13 slab, 2 slabs
SLAB2 = MO // 2              # 4 panels per w2 slab, 2 slabs

F32 = mybir.dt.float32
BF16 = mybir.dt.bfloat16
NP_BF16 = ml_dtypes.bfloat16


def build_program(C: int, repeats: int = 1, hw_loop: bool = False,
                  unroll: int = 35):
    """Build + compile the SPMD per-core program for capacity C.

    repeats > 1 re-runs the whole computation back-to-back inside one NEFF
    (identical output); used only for steady-state timing in test.py.
    """
    nc = bacc.Bacc(
        "TRN2", target_bir_lowering=False, debug=False, num_devices=E
    )
    xg_d = nc.dram_tensor("xg", [P, KD * C], BF16, kind="ExternalInput").ap()
    w13p_d = nc.dram_tensor(
        "w13p", [MP, P, PANW13], BF16, kind="ExternalInput"
    ).ap()
    w2p_d = nc.dram_tensor(
        "w2p", [MO, P, PANW2], BF16, kind="ExternalInput"
    ).ap()
    out_d = nc.dram_tensor(
        "outt", [P, MO * C], BF16, kind="ExternalOutput"
    ).ap()

    with tile.TileContext(nc) as tc, ExitStack() as ctx:
        resident = ctx.enter_context(tc.tile_pool(name="resident", bufs=1))
        psum = ctx.enter_context(tc.tile_pool(name="psum", bufs=2, space="PSUM"))
        spool = ctx.enter_context(tc.tile_pool(name="s", bufs=3))

        pools = (resident, psum, spool)

        # --- one-time PE warm-up, outside the timed loop body ---
        warm = resident.tile([P, 512], BF16, tag="warm")
        nc.gpsimd.memset(warm[:], 0.0)
        pwarm = psum.tile([P, 512], F32, tag="psg", bufs=2)
        for _ in range(8):
            nc.tensor.matmul(
                pwarm[:, :C], lhsT=warm[:, :P], rhs=warm[:, :C],
                start=True, stop=True,
            )
        sil_warm = spool.tile([P, 1], F32, tag="sil_warm")
        nc.scalar.activation(
            sil_warm[:], warm[:, :1], mybir.ActivationFunctionType.Silu
        )

        if hw_loop and repeats > 1:
            u = math.gcd(unroll, repeats) if repeats % unroll else unroll
            with tc.For_i(0, repeats // u, 1):
                # Post-barrier re-warm: dummy matmuls keep the PE busy (and
                # push HAM back to 2.4GHz) during the DMA re-issue window
                # right after the all-engine loop barrier.
                pw = psum.tile([P, 512], F32, tag="psg", bufs=2)
                for _ in range(16):
                    nc.tensor.matmul(
                        pw[:, :C], lhsT=warm[:, :P], rhs=warm[:, :C],
                        start=True, stop=True,
                    )
                for _ in range(u):
                    _emit_body(nc, tc, pools, xg_d, w13p_d, w2p_d, out_d, C)
        else:
            for _ in range(repeats):
                _emit_body(nc, tc, pools, xg_d, w13p_d, w2p_d, out_d, C)

    nc.compile()
    return nc


def _emit_body(nc, tc, pools, xg_d, w13p_d, w2p_d, out_d, C):
    resident, psum, spool = pools

    # --- all input DMAs, consumption order, single sync HWDGE queue ---
    xg = resident.tile([P, KD * C], BF16, tag="xg", bufs=2)
    nc.sync.dma_start(out=xg[:], in_=xg_d[:])
    w13t = []
    for p in range(MP):
        t = resident.tile([P, PANW13], BF16, tag=f"w13_{p}")
        nc.sync.dma_start(out=t[:], in_=w13p_d[p])
        w13t.append(t)
    w2t = []
    for mo in range(MO):
        t = resident.tile([P, PANW2], BF16, tag=f"w2_{mo}")
        nc.sync.dma_start(out=t[:], in_=w2p_d[mo])
        w2t.append(t)

    def w13_panel(p):
        return w13t[p][:, :]

    hT_all = resident.tile([P, KI * C], BF16, tag="hT_all")
    obuf = resident.tile([P, MO * C], BF16, tag="obuf", bufs=2)

    # --- GEMM1 + SiLU*up, one fused gate|up panel pair at a time ---
    for p in range(MP):
        wt = w13_panel(p)
        psg = psum.tile([P, 512], F32, tag="psg", bufs=2)
        psu = psum.tile([P, 512], F32, tag="psu", bufs=2)
        for k in range(KD):
            nc.tensor.matmul(
                psg[:, :C],
                lhsT=wt[:, k * 2 * P : k * 2 * P + P],
                rhs=xg[:, k * C : (k + 1) * C],
                start=(k == 0),
                stop=(k == KD - 1),
            )
        for k in range(KD):
            nc.tensor.matmul(
                psu[:, :C],
                lhsT=wt[:, k * 2 * P + P : (k + 1) * 2 * P],
                rhs=xg[:, k * C : (k + 1) * C],
                start=(k == 0),
                stop=(k == KD - 1),
            )
        sil = spool.tile([P, C], F32, tag="sil")
        nc.scalar.activation(
            sil[:], psg[:, :C], mybir.ActivationFunctionType.Silu
        )
        nc.vector.tensor_mul(
            hT_all[:, p * C : (p + 1) * C], sil[:], psu[:, :C]
        )

    # --- GEMM2: outT tile by tile into the staging buffer ---
    for mo in range(MO):
        w2 = w2t[mo]
        ps2 = psum.tile([P, 512], F32, tag="ps2", bufs=3)
        for ki in range(KI):
            nc.tensor.matmul(
                ps2[:, :C],
                lhsT=w2[:, ki * P : (ki + 1) * P],
                rhs=hT_all[:, ki * C : (ki + 1) * C],
                start=(ki == 0),
                stop=(ki == KI - 1),
            )
        nc.vector.tensor_copy(obuf[:, mo * C : (mo + 1) * C], ps2[:, :C])
    # one batched output store on the scalar queue
    nc.scalar.dma_start(out=out_d[:], in_=obuf[:])


def prepare_core_inputs(x, expert_indices, w13, w2):
    """Host-side routing + packing. Returns (in_maps, slot_lists, C)."""
    x = np.asarray(x)
    flat_e = np.asarray(expert_indices).reshape(-1).astype(np.int64)
    T = flat_e.shape[0]
    A = T // x.shape[0]
    slot_lists = [np.nonzero(flat_e == e)[0] for e in range(E)]
    max_n = max(1, max(len(s) for s in slot_lists))
    C = max(256, ((max_n + 3) // 4) * 4)

    w13 = np.asarray(w13)
    w2 = np.asarray(w2)
    in_maps = []
    for e in range(E):
        slots = slot_lists[e]
        tok = slots // A
        xg = np.zeros((D, C), dtype=NP_BF16)
        if len(tok):
            xg[:, : len(tok)] = x[tok].T.astype(NP_BF16)
        # SBUF image: [P, KD*C] — row p holds xgT[k*128+p, :] for k=0..KD-1
        xg = np.ascontiguousarray(
            xg.reshape(KD, P, C).transpose(1, 0, 2).reshape(P, KD * C)
        )

        w13t = w13[e].T.astype(NP_BF16)  # [D, 2I]
        a = w13t.reshape(KD, P, 2 * MP, P)
        w13p = np.concatenate([a[:, :, :MP, :], a[:, :, MP:, :]], axis=-1)
        # fused gate|up image per pair-panel: [MP, P, KD*2P]
        w13p = np.ascontiguousarray(
            w13p.transpose(2, 1, 0, 3).reshape(MP, P, PANW13)
        )


        w2t = w2[e].T.astype(NP_BF16)  # [I, D]
        b = w2t.reshape(KI, P, MO, P)
        # per out-panel image: [MO, P, KI*P]
        w2p = np.ascontiguousarray(
            b.transpose(2, 1, 0, 3).reshape(MO, P, PANW2)
        )
        in_maps.append({"xg": xg, "w13p": w13p, "w2p": w2p})
    return in_maps, slot_lists, C


def assemble_output(results, slot_lists, T, dtype):
    out = np.zeros((T, D), dtype=dtype)
    for e in range(E):
        slots = slot_lists[e]
        if len(slots) == 0:
            continue
        flat = np.asarray(results[e]["outt"])  # [P, MO*C]
        Ccap = flat.shape[1] // MO
        outt = flat.reshape(P, MO, Ccap).transpose(1, 0, 2).reshape(D, Ccap)
        out[slots] = outt[:, : len(slots)].T.astype(dtype)
    return out


def kernel(x, expert_indices, w13, w2):
    in_maps, slot_lists, C = prepare_core_inputs(x, expert_indices, w13, w2)
    if C > 512:
        # Pathological imbalance: PSUM limits one pass to 512 tokens/expert.
        # Split each expert's token list into <=512-sized chunks and run the
        # fixed-capacity program once per chunk round.
        T = np.asarray(expert_indices).size
        out = np.zeros((T, D), dtype=np.asarray(x).dtype)
        chunked = [
            [s[i : i + 512] for i in range(0, max(len(s), 1), 512)]
            for s in slot_lists
        ]
        rounds = max(len(c) for c in chunked)
        for r in range(rounds):
            sub_slots = [
                c[r] if r < len(c) else np.zeros(0, dtype=np.int64)
                for c in chunked
            ]
            flat = np.full(T, -1, dtype=np.int64)
            for e, s in enumerate(sub_slots):
                flat[s] = e
            sub_maps, sub_lists, subC = prepare_core_inputs(
                x, flat.reshape(np.asarray(expert_indices).shape), w13, w2
            )
            nc = build_program(subC)
            res = _run_with_retry(nc, sub_maps)
            part = assemble_output(
                res.results, sub_lists, T, np.asarray(x).dtype
            )
            mask = flat >= 0
            out[mask] = part[mask]
        return out
    nc = build_program(C)
    res = _run_with_retry(nc, in_maps)
    T = np.asarray(expert_indices).size
    return assemble_output(res.results, slot_lists, T, np.asarray(x).dtype)


def _run_with_retry(nc, in_maps, attempts=3):
    last_err = None
    for _ in range(attempts):
        try:
            return run_bass_kernel_spmd(nc, in_maps, core_ids=list(range(E)))
        except Exception as exc:  # intermittent NRT exec-unit wedge: retry
            last_err = exc
    raise last_err


# revision 10
# speedup vs baseline: 1.0342x; 1.0342x over previous
"""Trainium2 Bass kernel for nn_ConditionalFeedForward (MoE top-2 FFN).

Strategy: expert-parallel across the 8 NeuronCores — expert e lives on core e.
Routing/gather/scatter (pure index bookkeeping) happens on the host; all FLOPs
(both GEMMs + SiLU) run on device.

Per core (expert e), with C = per-expert token capacity:
    h13T = w13[e] @ xgT          # [2I, C], accumulation over D in PSUM
    hT   = silu(gate) * up       # [I, C]
    outT = w2[e] @ hT            # [D, C]
Everything is kept transposed ([feature, token]) so both GEMMs use the weight
as the stationary operand and never need an on-device transpose.

Performance notes (from NTFF profiling):
  - PSUM tiles are padded to full 2KB banks: two accumulation chains sharing
    one bank cost ~26ns on every matmul issue (144 vs the 118ns floor).
  - All weights are SBUF-resident (~14MB).  Weight DMAs are issued up-front
    in consumption order on the sync HWDGE queue; loop-carried WAR deps
    (a tile is reloadable once the previous rep finished reading it)
    self-regulate into a full iteration of prefetch.
  - The 8 per-tile output DMAs are batched into one [P, MO*C] store: tiny
    552B-per-partition descriptors otherwise pollute the SDMA engines'
    packet round-robin.
  - Output DMA rides the scalar HWDGE queue so it never blocks weight
    prefetch; output is cast to bf16 by the DVE copy.
  - PSUM: psg/psu double-buffered, ps2 triple, output staging double -
    the out-DMA completion latency (~2-3us HBM write receipt) otherwise
    backpressures GEMM2 via the PSUM-bank recycle chain.
  - For_i back-edge costs ~6-13us (all-engine barrier + DMA re-issue + PE
    HAM re-throttle), so `unroll` bodies run per hw-loop iteration and a
    dummy-matmul warm block covers the post-barrier refill window.
"""

import math
from contextlib import ExitStack

import ml_dtypes
import numpy as np

import concourse.bass as bass
import concourse.mybir as mybir
import concourse.tile as tile
from concourse import bacc
from concourse.bass_utils import run_bass_kernel_spmd

# Problem shape (hardcoded per harness contract).
E = 8          # experts == cores
D = 1024       # model dim
I = 2048       # intermediate dim
I2 = 2 * I     # fused gate+up rows of w13
P = 128        # SBUF partitions
KD = D // P    # 8 k-tiles over D
MP = I // P    # 16 gate/up pair panels
MO = D // P    # 8 output row tiles
KI = I // P    # 16 k-tiles over I

PANW13 = KD * 2 * P      # w13 pair-panel width in elements (4096)
PANW2 = KI * P           # w2 out-panel width in elements (2048)

F32 = mybir.dt.float32
BF16 = mybir.dt.bfloat16
NP_BF16 = ml_dtypes.bfloat16


def build_program(C: int, repeats: int = 1, hw_loop: bool = False,
                  unroll: int = 35):
    """Build + compile the SPMD per-core program for capacity C.

    repeats > 1 re-runs the whole computation back-to-back inside one NEFF
    (identical output); used only for steady-state timing in test.py.
    """
    nc = bacc.Bacc(
        "TRN2", target_bir_lowering=False, debug=False, num_devices=E
    )
    xg_d = nc.dram_tensor("xg", [P, KD * C], BF16, kind="ExternalInput").ap()
    w13p_d = nc.dram_tensor(
        "w13p", [MP, P, PANW13], BF16, kind="ExternalInput"
    ).ap()
    w2p_d = nc.dram_tensor(
        "w2p", [MO, P, PANW2], BF16, kind="ExternalInput"
    ).ap()
    out_d = nc.dram_tensor(
        "outt", [P, MO * C], BF16, kind="ExternalOutput"
    ).ap()

    with tile.TileContext(nc) as tc, ExitStack() as ctx:
        resident = ctx.enter_context(tc.tile_pool(name="resident", bufs=1))
        psum = ctx.enter_context(tc.tile_pool(name="psum", bufs=2, space="PSUM"))
        spool = ctx.enter_context(tc.tile_pool(name="s", bufs=3))

        pools = (resident, psum, spool)

        # --- one-time PE warm-up, outside the timed loop body ---
        warm = resident.tile([P, 512], BF16, tag="warm")
        nc.gpsimd.memset(warm[:], 0.0)
        pwarm = psum.tile([P, 512], F32, tag="psg", bufs=2)
        for _ in range(8):
            nc.tensor.matmul(
                pwarm[:, :C], lhsT=warm[:, :P], rhs=warm[:, :C],
                start=True, stop=True,
            )
        sil_warm = spool.tile([P, 1], F32, tag="sil_warm")
        nc.scalar.activation(
            sil_warm[:], warm[:, :1], mybir.ActivationFunctionType.Silu
        )

        if hw_loop and repeats > 1:
            u = math.gcd(unroll, repeats) if repeats % unroll else unroll
            with tc.For_i(0, repeats // u, 1):
                # Post-barrier re-warm: dummy matmuls keep the PE busy (and
                # push HAM back to 2.4GHz) during the DMA re-issue window
                # right after the all-engine loop barrier.
                pw = psum.tile([P, 512], F32, tag="psg", bufs=2)
                for _ in range(16):
                    nc.tensor.matmul(
                        pw[:, :C], lhsT=warm[:, :P], rhs=warm[:, :C],
                        start=True, stop=True,
                    )
                for _ in range(u):
                    _emit_body(nc, tc, pools, xg_d, w13p_d, w2p_d, out_d, C)
        else:
            for _ in range(repeats):
                _emit_body(nc, tc, pools, xg_d, w13p_d, w2p_d, out_d, C)

    nc.compile()
    return nc


def _emit_body(nc, tc, pools, xg_d, w13p_d, w2p_d, out_d, C):
    resident, psum, spool = pools

    # --- all input DMAs, consumption order, single sync HWDGE queue ---
    xg = resident.tile([P, KD * C], BF16, tag="xg", bufs=2)
    nc.sync.dma_start(out=xg[:], in_=xg_d[:])
    w13t = []
    for p in range(MP):
        t = resident.tile([P, PANW13], BF16, tag=f"w13_{p}")
        nc.sync.dma_start(out=t[:], in_=w13p_d[p])
        w13t.append(t)
    w2t = []
    for mo in range(MO):
        t = resident.tile([P, PANW2], BF16, tag=f"w2_{mo}")
        nc.sync.dma_start(out=t[:], in_=w2p_d[mo])
        w2t.append(t)

    def w13_panel(p):
        return w13t[p][:, :]

    hT_all = resident.tile([P, KI * C], BF16, tag="hT_all")
    obuf = resident.tile([P, MO * C], BF16, tag="obuf", bufs=2)

    # --- GEMM1 + SiLU*up, one fused gate|up panel pair at a time ---
    for p in range(MP):
        wt = w13_panel(p)
        psg = psum.tile([P, 512], F32, tag="psg", bufs=2)
        psu = psum.tile([P, 512], F32, tag="psu", bufs=2)
        for k in range(KD):
            nc.tensor.matmul(
                psg[:, :C],
                lhsT=wt[:, k * 2 * P : k * 2 * P + P],
                rhs=xg[:, k * C : (k + 1) * C],
                start=(k == 0),
                stop=(k == KD - 1),
            )
        for k in range(KD):
            nc.tensor.matmul(
                psu[:, :C],
                lhsT=wt[:, k * 2 * P + P : (k + 1) * 2 * P],
                rhs=xg[:, k * C : (k + 1) * C],
                start=(k == 0),
                stop=(k == KD - 1),
            )
        sil = spool.tile([P, C], F32, tag="sil")
        nc.scalar.activation(
            sil[:], psg[:, :C], mybir.ActivationFunctionType.Silu
        )
        nc.vector.tensor_mul(
            hT_all[:, p * C : (p + 1) * C], sil[:], psu[:, :C]
        )

    # --- GEMM2: outT tile by tile into the staging buffer ---
    for mo in range(MO):
        w2 = w2t[mo]
        ps2 = psum.tile([P, 512], F32, tag="ps2", bufs=3)
        for ki in range(KI):
            nc.tensor.matmul(
                ps2[:, :C],
                lhsT=w2[:, ki * P : (ki + 1) * P],
                rhs=hT_all[:, ki * C : (ki + 1) * C],
                start=(ki == 0),
                stop=(ki == KI - 1),
            )
        nc.vector.tensor_copy(obuf[:, mo * C : (mo + 1) * C], ps2[:, :C])
    # one batched output store on the scalar queue
    nc.scalar.dma_start(out=out_d[:], in_=obuf[:])


def prepare_core_inputs(x, expert_indices, w13, w2):
    """Host-side routing + packing. Returns (in_maps, slot_lists, C)."""
    x = np.asarray(x)
    flat_e = np.asarray(expert_indices).reshape(-1).astype(np.int64)
    T = flat_e.shape[0]
    A = T // x.shape[0]
    slot_lists = [np.nonzero(flat_e == e)[0] for e in range(E)]
    max_n = max(1, max(len(s) for s in slot_lists))
    C = max(256, ((max_n + 3) // 4) * 4)

    w13 = np.asarray(w13)
    w2 = np.asarray(w2)
    in_maps = []
    for e in range(E):
        slots = slot_lists[e]
        tok = slots // A
        xg = np.zeros((D, C), dtype=NP_BF16)
        if len(tok):
            xg[:, : len(tok)] = x[tok].T.astype(NP_BF16)
        # SBUF image: [P, KD*C] — row p holds xgT[k*128+p, :] for k=0..KD-1
        xg = np.ascontiguousarray(
            xg.reshape(KD, P, C).transpose(1, 0, 2).reshape(P, KD * C)
        )

        w13t = w13[e].T.astype(NP_BF16)  # [D, 2I]
        a = w13t.reshape(KD, P, 2 * MP, P)
        w13p = np.concatenate([a[:, :, :MP, :], a[:, :, MP:, :]], axis=-1)
        # fused gate|up image per pair-panel: [MP, P, KD*2P]
        w13p = np.ascontiguousarray(
            w13p.transpose(2, 1, 0, 3).reshape(MP, P, PANW13)
        )


        w2t = w2[e].T.astype(NP_BF16)  # [I, D]
        b = w2t.reshape(KI, P, MO, P)
        # per out-panel image: [MO, P, KI*P]
        w2p = np.ascontiguousarray(
            b.transpose(2, 1, 0, 3).reshape(MO, P, PANW2)
        )
        in_maps.append({"xg": xg, "w13p": w13p, "w2p": w2p})
    return in_maps, slot_lists, C


def assemble_output(results, slot_lists, T, dtype):
    out = np.zeros((T, D), dtype=dtype)
    for e in range(E):
        slots = slot_lists[e]
        if len(slots) == 0:
            continue
        flat = np.asarray(results[e]["outt"])  # [P, MO*C]
        Ccap = flat.shape[1] // MO
        outt = flat.reshape(P, MO, Ccap).transpose(1, 0, 2).reshape(D, Ccap)
        out[slots] = outt[:, : len(slots)].T.astype(dtype)
    return out


def kernel(x, expert_indices, w13, w2):
    in_maps, slot_lists, C = prepare_core_inputs(x, expert_indices, w13, w2)
    if C > 512:
        # Pathological imbalance: PSUM limits one pass to 512 tokens/expert.
        # Split each expert's token list into <=512-sized chunks and run the
        # fixed-capacity program once per chunk round.
        T = np.asarray(expert_indices).size
        out = np.zeros((T, D), dtype=np.asarray(x).dtype)
        chunked = [
            [s[i : i + 512] for i in range(0, max(len(s), 1), 512)]
            for s in slot_lists
        ]
        rounds = max(len(c) for c in chunked)
        for r in range(rounds):
            sub_slots = [
                c[r] if r < len(c) else np.zeros(0, dtype=np.int64)
                for c in chunked
            ]
            flat = np.full(T, -1, dtype=np.int64)
            for e, s in enumerate(sub_slots):
                flat[s] = e
            sub_maps, sub_lists, subC = prepare_core_inputs(
                x, flat.reshape(np.asarray(expert_indices).shape), w13, w2
            )
            nc = build_program(subC)
            res = _run_with_retry(nc, sub_maps)
            part = assemble_output(
                res.results, sub_lists, T, np.asarray(x).dtype
            )
            mask = flat >= 0
            out[mask] = part[mask]
        return out
    nc = build_program(C)
    res = _run_with_retry(nc, in_maps)
    T = np.asarray(expert_indices).size
    return assemble_output(res.results, slot_lists, T, np.asarray(x).dtype)


def _run_with_retry(nc, in_maps, attempts=3):
    last_err = None
    for _ in range(attempts):
        try:
            return run_bass_kernel_spmd(nc, in_maps, core_ids=list(range(E)))
        except Exception as exc:  # intermittent NRT exec-unit wedge: retry
            last_err = exc
    raise last_err
